# revision 1
# baseline (speedup 1.0000x reference)
"""Bass/Trainium2 kernel for the DGPE relaxation RHS on a 192^3 periodic lattice.

The nn_id* inputs are the deterministic 6-neighbor roll indices of the
lattice, so the gathers are implemented as stencil shifts.  The lattice is
sharded along axis 0 across 8 NeuronCores (24 planes + 2 halo planes each,
sliced host-side).  Within a core, partition = (k-block, j-block) = 8 x 16,
each partition holding a (24 x 12 x 24) sub-brick stored with j/k halo
strips so every neighbor access is a plain access-pattern offset.
"""

import numpy as np

L = 192
N = L ** 3
NCORES = 8
CH = L // NCORES            # 24 planes (axis 0) per core
KH, JB = 8, 16              # partition grid: p = kh*JB + jb
JW = L // JB                # 12 j's per partition
KW = L // KH                # 24 k's per partition
IH = CH + 2                 # 26 planes incl. axis-0 halo
FJ = JW + 2                 # 14 incl. j halo strips
FK = KW + 2                 # 26 incl. k halo strips
PLF = FJ * FK               # padded plane free size (364)
FIN = IH * PLF
PF = JW * KW                # compact plane free size (288)
FOUT = CH * PF
T = 8                       # planes per compute tile
NT = CH // T
TF = T * PF

_STATE = {}


# ---------------------------------------------------------------- host side

def _shard_halo(v3pad):
    """(194,194,194) wrap-padded -> (8, 128, FIN) per-core images."""
    s0, s1, s2 = v3pad.strides
    v = np.lib.stride_tricks.as_strided(
        v3pad,
        shape=(NCORES, KH, JB, IH, FJ, FK),
        strides=(CH * s0, KW * s2, JW * s1, s0, s1, s2),
    )
    return np.ascontiguousarray(v).reshape(NCORES, 128, FIN)


def _shard_compact(v3):
    """(192,192,192) -> (8, 128, CH, PF) per-core compact images."""
    s0, s1, s2 = v3.strides
    v = np.lib.stride_tricks.as_strided(
        v3,
        shape=(NCORES, KH, JB, CH, JW, KW),
        strides=(CH * s0, KW * s2, JW * s1, s0, s1, s2),
    )
    return np.ascontiguousarray(v).reshape(NCORES, 128, CH, PF)


def _unshard_compact(per_core):
    """(8, 128, CH*PF) -> (192,192,192)."""
    out3 = np.empty((L, L, L), np.float32)
    s0, s1, s2 = out3.strides
    w = np.lib.stride_tricks.as_strided(
        out3,
        shape=(NCORES, KH, JB, CH, JW, KW),
        strides=(CH * s0, KW * s2, JW * s1, s0, s1, s2),
    )
    w[:] = per_core.reshape(NCORES, KH, JB, CH, JW, KW)
    return out3


def _is_const(a):
    a = np.asarray(a)
    return bool(a.size) and bool(np.all(a == a.flat[0]))


def _rolls_ok(nn_idx_1, nn_idx_2, nn_idy_1, nn_idy_2, nn_idz_1, nn_idz_2):
    """Spot-check that the index arrays are the periodic roll stencil."""
    rng = np.random.default_rng(12345)
    f = rng.integers(0, N, size=4096)
    i, r = np.divmod(f, L * L)
    j, k = np.divmod(r, L)

    def flat(ii, jj, kk):
        return (ii % L) * L * L + (jj % L) * L + (kk % L)

    checks = [
        (nn_idx_1, flat(i - 1, j, k)), (nn_idx_2, flat(i + 1, j, k)),
        (nn_idy_1, flat(i, j - 1, k)), (nn_idy_2, flat(i, j + 1, k)),
        (nn_idz_1, flat(i, j, k - 1)), (nn_idz_2, flat(i, j, k + 1)),
    ]
    for arr, want in checks:
        if not np.array_equal(np.asarray(arr)[f], want):
            return False
    return True


def _numpy_fallback(y, J, anisotropy, gamma, h_dis_x, h_dis_y, beta,
                    e_disorder, idx):
    """Exact reference math in numpy (used only if structure checks fail)."""
    x, p = y[:N], y[N:]

    def stencil(v):
        return J * (v[idx[0]] + v[idx[1]] + v[idx[2]] + v[idx[3]]
                    + anisotropy * (v[idx[4]] + v[idx[5]]))

    xL = stencil(x)
    yL = stencil(p)
    r2 = x * x + p * p
    cross = xL * p - yL * x
    dx = gamma * p * cross + e_disorder * p - yL + h_dis_y + beta * r2 * p
    dp = -gamma * x * cross - e_disorder * x + xL - h_dis_x - beta * r2 * x
    return np.concatenate([dx, dp]).astype(np.float32)


# -------------------------------------------------------------- device side

def _build_nc():
    from concourse import bacc
    import concourse.mybir as mybir
    from concourse.mybir import AluOpType as Op
    from concourse.tile import TileContext, add_dep_helper

    ActF = mybir.ActivationFunctionType
    f32 = mybir.dt.float32

    nc = bacc.Bacc("TRN2", target_bir_lowering=False, debug=False,
                   enable_asserts=False, num_devices=NCORES)
    x_in = nc.dram_tensor("x_in", [128, FIN], f32, kind="ExternalInput").ap()
    p_in = nc.dram_tensor("p_in", [128, FIN], f32, kind="ExternalInput").ap()
    # packed per-tile coefficients: [e_disorder | h_dis_x | h_dis_y]
    cf_in = nc.dram_tensor("cf_in", [128, NT, 3, TF], f32, kind="ExternalInput").ap()
    cst_in = nc.dram_tensor("cst_in", [128, 8], f32, kind="ExternalInput").ap()
    dx_out = nc.dram_tensor("dx_out", [128, FOUT], f32, kind="ExternalOutput").ap()
    dp_out = nc.dram_tensor("dp_out", [128, FOUT], f32, kind="ExternalOutput").ap()

    with TileContext(nc) as tc:
        with (
            tc.tile_pool(name="persist", bufs=1) as pers,
            tc.tile_pool(name="state", bufs=2) as sp,
            tc.tile_pool(name="coef", bufs=1) as cp,
            tc.tile_pool(name="outs", bufs=2) as op_,
            tc.tile_pool(name="ubuf", bufs=2) as up,
            tc.tile_pool(name="tmp", bufs=1) as tp,
        ):
            cst = pers.tile([128, 8], f32, name="cst")
            ANIS = cst[:, 0:1]   # anisotropy
            GJ = cst[:, 1:2]     # gamma * J
            BET = cst[:, 2:3]    # beta
            JC = cst[:, 3:4]     # J
            NJC = cst[:, 4:5]    # -J

            for t in range(NT):
                i0 = t * T
                f0 = i0 * PF

                xt = sp.tile([128, (T + 2) * PLF], f32, tag="xt", name=f"xt{t}")
                if t == 0:
                    Hx = (T + 2) * PLF // 2
                    nc.sync.dma_start(xt[:, :Hx], x_in[:, :Hx])
                    nc.sync.dma_start(xt[:, Hx:], x_in[:, Hx:(T + 2) * PLF])
                else:
                    nc.sync.dma_start(xt[:], x_in[:, i0 * PLF:(i0 + T + 2) * PLF])
                pt = sp.tile([128, (T + 2) * PLF], f32, tag="pt", name=f"pt{t}")
                nc.sync.dma_start(pt[:], p_in[:, i0 * PLF:(i0 + T + 2) * PLF])
                if t == 0:
                    nc.sync.dma_start(cst[:], cst_in)
                ct = cp.tile([128, 3, TF], f32, tag="ct", name=f"ct{t}")
                nc.sync.dma_start(ct[:], cf_in[:, t])
                ed, hx, hy = ct[:, 0, :], ct[:, 1, :], ct[:, 2, :]

                def sl(img, di, dj, dk):
                    v = img[:].rearrange("q (i j k) -> q i j k",
                                         i=T + 2, j=FJ, k=FK)
                    return v[:, 1 + di: 1 + T + di,
                             1 + dj: 1 + JW + dj, 1 + dk: 1 + KW + dk]

                xc = sl(xt, 0, 0, 0)
                pc = sl(pt, 0, 0, 0)

                def v4(tile):
                    return tile[:].rearrange("q (i j k) -> q i j k",
                                             i=T, j=JW, k=KW)

                S1 = tp.tile([128, TF], f32, tag="S1", name=f"S1_{t}")
                S2 = tp.tile([128, TF], f32, tag="S2", name=f"S2_{t}")
                S3 = tp.tile([128, TF], f32, tag="S3", name=f"S3_{t}")
                S4 = tp.tile([128, TF], f32, tag="S4", name=f"S4_{t}")
                S5 = tp.tile([128, TF], f32, tag="S5", name=f"S5_{t}")

                # ---- x stencil: vx = (i-sum) + (j-sum) + anis*(k-sum)
                nc.vector.tensor_add(v4(S1), sl(xt, -1, 0, 0), sl(xt, 1, 0, 0))
                nc.vector.tensor_add(v4(S2), sl(xt, 0, -1, 0), sl(xt, 0, 1, 0))
                nc.vector.tensor_add(v4(S5), sl(xt, 0, 0, -1), sl(xt, 0, 0, 1))

                # ---- r2 = x^2 + p^2: squares on ACT (own SBUF ports, runs
                # alongside the DVE stencil work).  GpSimd compute and DMA
                # accumulates are avoided: the former locks the shared SBUF
                # port (~3x DVE slowdown), the latter proved fragile.
                nc.scalar.activation(v4(S3), xc, ActF.Square)
                nc.scalar.activation(v4(S4), pc, ActF.Square)
                nc.vector.tensor_add(S3[:], S3[:], S4[:])            # r2

                nc.vector.scalar_tensor_tensor(S5[:], S5[:], ANIS, S2[:], Op.mult, Op.add)
                nc.vector.tensor_add(S1[:], S5[:], S1[:])            # vx

                # ---- p stencil
                nc.vector.tensor_add(v4(S2), sl(pt, -1, 0, 0), sl(pt, 1, 0, 0))
                nc.vector.tensor_add(v4(S5), sl(pt, 0, -1, 0), sl(pt, 0, 1, 0))
                nc.vector.tensor_add(v4(S4), sl(pt, 0, 0, -1), sl(pt, 0, 0, 1))
                nc.vector.scalar_tensor_tensor(S4[:], S4[:], ANIS, S5[:], Op.mult, Op.add)
                nc.vector.tensor_add(S2[:], S4[:], S2[:])            # vy

                # ---- cross_raw = vx*p - vy*x
                nc.vector.tensor_mul(v4(S4), v4(S2), xc)             # w2 = vy*x
                nc.vector.tensor_mul(v4(S5), v4(S1), pc)             # w1 = vx*p
                nc.vector.tensor_sub(S4[:], S5[:], S4[:])            # cross_raw

                # ---- s2 = (gamma*J)*cross_raw + e_dis + beta*r2
                nc.vector.scalar_tensor_tensor(S4[:], S4[:], GJ, ed, Op.mult, Op.add)
                nc.vector.scalar_tensor_tensor(S4[:], S3[:], BET, S4[:], Op.mult, Op.add)

                # ---- dx = p*s2 + (h_y - J*vy)
                dxo = op_.tile([128, TF], f32, tag="dxo", name=f"dxo{t}")
                nc.vector.tensor_mul(v4(dxo), pc, v4(S4))            # t1
                if t < NT - 1:
                    # final add rides the store: plain store of t1, then a
                    # CCE accumulate of u1 into the same DRAM range (split
                    # to respect the 2048-elem CCE descriptor limit)
                    u1t = up.tile([128, TF], f32, tag="u1t", name=f"u1t{t}")
                    nc.vector.scalar_tensor_tensor(u1t[:], S2[:], NJC, hy, Op.mult, Op.add)  # u1
                    st1 = nc.sync.dma_start(dx_out[:, f0:f0 + TF], dxo[:])
                    Hh = TF // 2
                    for lo, hi in ((0, Hh), (Hh, TF)):
                        a = nc.gpsimd.dma_start(dx_out[:, f0 + lo:f0 + hi],
                                                u1t[:, lo:hi], accum_op=Op.add)
                        add_dep_helper(a.ins, st1.ins, reason="dram RMW after t1 store")
                else:
                    nc.vector.scalar_tensor_tensor(S5[:], S2[:], NJC, hy, Op.mult, Op.add)  # u1
                    nc.vector.tensor_add(dxo[:], dxo[:], S5[:])
                    nc.sync.dma_start(dx_out[:, f0:f0 + TF], dxo[:])

                # ---- dp = (J*vx - h_x) - x*s2
                dpo = op_.tile([128, TF], f32, tag="dpo", name=f"dpo{t}")
                if t == NT - 1:
                    nc.vector.scalar_tensor_tensor(S1[:], S1[:], JC, hx, Op.mult, Op.subtract)  # u2
                    nc.vector.tensor_mul(v4(dpo), xc, v4(S4))        # t2
                    Hh = TF // 2
                    nc.vector.tensor_sub(dpo[:, :Hh], S1[:, :Hh], dpo[:, :Hh])
                    nc.sync.dma_start(dp_out[:, f0:f0 + Hh], dpo[:, :Hh])
                    nc.vector.tensor_sub(dpo[:, Hh:], S1[:, Hh:], dpo[:, Hh:])
                    nc.sync.dma_start(dp_out[:, f0 + Hh:f0 + TF], dpo[:, Hh:])
                else:
                    # dp = u2 + (-x*s2): store u2, negate s2 in place (2x
                    # single-src op, after t1 consumed it), then accumulate
                    u2t = up.tile([128, TF], f32, tag="u2t", name=f"u2t{t}")
                    nc.vector.scalar_tensor_tensor(u2t[:], S1[:], JC, hx, Op.mult, Op.subtract)  # u2
                    st2 = nc.sync.dma_start(dp_out[:, f0:f0 + TF], u2t[:])
                    nc.vector.tensor_scalar_mul(S4[:], S4[:], -1.0)  # -s2
                    nc.vector.tensor_mul(v4(dpo), xc, v4(S4))        # -t2
                    Hh = TF // 2
                    for lo, hi in ((0, Hh), (Hh, TF)):
                        a = nc.gpsimd.dma_start(dp_out[:, f0 + lo:f0 + hi],
                                                dpo[:, lo:hi], accum_op=Op.add)
                        add_dep_helper(a.ins, st2.ins, reason="dram RMW after u2 store")

    nc.compile()
    return nc


def _get_nc():
    if "nc" not in _STATE:
        _STATE["nc"] = _build_nc()
    return _STATE["nc"]


def _run(in_maps, trace=False, trace_cores=None):
    from concourse.bass_utils import run_bass_kernel_spmd
    if trace:
        # the agent image's antenv lacks axon_hooks; wire the NTFF hook
        import sys as _sys
        import types as _types
        if "antenv.axon_hooks" not in _sys.modules:
            try:
                import trn_agent_boot.trn_boot as _tb
                _hook = _tb._ntff_profile_via_ctypes('/opt/axon/libaxon_pjrt.so')
                _mod = _types.ModuleType("antenv.axon_hooks")
                _mod.get_axon_ntff_profile_hook = lambda: _hook
                _sys.modules["antenv.axon_hooks"] = _mod
            except Exception:
                pass
    return run_bass_kernel_spmd(
        _get_nc(), in_maps, core_ids=list(range(NCORES)),
        trace=trace, trace_cores=trace_cores,
    )


def prepare_in_maps(y, anis_v, gamma_v, beta_v, j_v, h_dis_x, h_dis_y,
                    e_disorder):
    """Host-side sharding: build the 8 per-core input maps."""
    x3 = np.ascontiguousarray(y[:N], np.float32).reshape(L, L, L)
    p3 = np.ascontiguousarray(y[N:], np.float32).reshape(L, L, L)
    xs = _shard_halo(np.pad(x3, 1, mode="wrap"))
    ps = _shard_halo(np.pad(p3, 1, mode="wrap"))
    eds = _shard_compact(np.ascontiguousarray(e_disorder, np.float32).reshape(L, L, L))
    hxs = _shard_compact(np.ascontiguousarray(h_dis_x, np.float32).reshape(L, L, L))
    hys = _shard_compact(np.ascontiguousarray(h_dis_y, np.float32).reshape(L, L, L))
    # pack per-tile coefficient blocks: [NT, 3, TF]
    cf = np.stack([eds.reshape(NCORES, 128, NT, TF),
                   hxs.reshape(NCORES, 128, NT, TF),
                   hys.reshape(NCORES, 128, NT, TF)], axis=3)
    cf = np.ascontiguousarray(cf)          # (8, 128, NT, 3, TF)
    cst = np.zeros((128, 8), np.float32)
    cst[:, 0] = anis_v
    cst[:, 1] = gamma_v * j_v
    cst[:, 2] = beta_v
    cst[:, 3] = j_v
    cst[:, 4] = -j_v
    return [
        {"x_in": xs[c], "p_in": ps[c], "cf_in": cf[c], "cst_in": cst}
        for c in range(NCORES)
    ]


def assemble_output(results):
    """Per-core device outputs -> full (2N,) float32 array."""
    dxs = np.stack([results[c]["dx_out"] for c in range(NCORES)])
    dps = np.stack([results[c]["dp_out"] for c in range(NCORES)])
    dx3 = _unshard_compact(dxs)
    dp3 = _unshard_compact(dps)
    return np.concatenate([dx3.reshape(-1), dp3.reshape(-1)])


def kernel(t, y, J, anisotropy, gamma, h_dis_x, h_dis_y, beta, e_disorder,
           nn_idx_1, nn_idx_2, nn_idy_1, nn_idy_2, nn_idz_1, nn_idz_2):
    y = np.asarray(y, np.float32)
    J = np.asarray(J, np.float32)
    anisotropy = np.asarray(anisotropy, np.float32)
    gamma = np.asarray(gamma, np.float32)
    beta = np.asarray(beta, np.float32)
    h_dis_x = np.asarray(h_dis_x, np.float32)
    h_dis_y = np.asarray(h_dis_y, np.float32)
    e_disorder = np.asarray(e_disorder, np.float32)

    ok = (y.shape == (2 * N,)
          and _is_const(J) and _is_const(anisotropy)
          and _is_const(gamma) and _is_const(beta)
          and _rolls_ok(nn_idx_1, nn_idx_2, nn_idy_1, nn_idy_2,
                        nn_idz_1, nn_idz_2))
    if not ok:
        idx = [np.asarray(a) for a in (nn_idx_1, nn_idx_2, nn_idy_1,
                                       nn_idy_2, nn_idz_1, nn_idz_2)]
        return _numpy_fallback(y, J, anisotropy, gamma, h_dis_x, h_dis_y,
                               beta, e_disorder, idx)

    in_maps = prepare_in_maps(
        y, float(anisotropy.flat[0]), float(gamma.flat[0]),
        float(beta.flat[0]), float(J.flat[0]), h_dis_x, h_dis_y, e_disorder)
    res = _run(in_maps, trace=False)
    return assemble_output(res.results)



# revision 3
# speedup vs baseline: 1.8585x; 1.8585x over previous
"""Bass/Trainium2 kernel for the DGPE relaxation RHS on a 192^3 periodic lattice.

The nn_id* inputs are the deterministic 6-neighbor roll indices of the
lattice, so the gathers are implemented as stencil shifts.  The lattice is
sharded along axis 0 across 8 NeuronCores (24 planes + 2 halo planes each,
sliced host-side).  Within a core, partition = (k-block, j-block) = 8 x 16,
each partition holding a (24 x 12 x 24) sub-brick stored with j/k halo
strips so every neighbor access is a plain access-pattern offset.

v2: all device data is fp16 (DVE 2x mode, half DMA traffic).  The full
6-neighbor stencil runs on the idle PE as identity-weight matmuls over
shifted views accumulating in PSUM (J and J*anisotropy on the diagonals),
the squares / psum->sbuf copies / gamma scale run on ACT, leaving DVE
only the 12 irreducible two-source elementwise ops.
"""

import numpy as np

L = 192
N = L ** 3
NCORES = 8
CH = L // NCORES            # 24 planes (axis 0) per core
KH, JB = 8, 16              # partition grid: p = kh*JB + jb
JW = L // JB                # 12 j's per partition
KW = L // KH                # 24 k's per partition
IH = CH + 2                 # 26 planes incl. axis-0 halo
FJ = JW + 2                 # 14 incl. j halo strips
FK = KW + 2                 # 26 incl. k halo strips
PLF = FJ * FK               # padded plane free size (364)
FIN = IH * PLF
PF = JW * KW                # compact plane free size (288)
FOUT = CH * PF
T = 8                       # planes per compute tile
NT = CH // T
TF = T * PF

_STATE = {}


# ---------------------------------------------------------------- host side

def _shard_halo(v3pad):
    """(194,194,194) wrap-padded -> (8, 128, FIN) per-core images."""
    s0, s1, s2 = v3pad.strides
    v = np.lib.stride_tricks.as_strided(
        v3pad,
        shape=(NCORES, KH, JB, IH, FJ, FK),
        strides=(CH * s0, KW * s2, JW * s1, s0, s1, s2),
    )
    return np.ascontiguousarray(v).reshape(NCORES, 128, FIN)


def _shard_compact(v3):
    """(192,192,192) -> (8, 128, CH, PF) per-core compact images."""
    s0, s1, s2 = v3.strides
    v = np.lib.stride_tricks.as_strided(
        v3,
        shape=(NCORES, KH, JB, CH, JW, KW),
        strides=(CH * s0, KW * s2, JW * s1, s0, s1, s2),
    )
    return np.ascontiguousarray(v).reshape(NCORES, 128, CH, PF)


def _unshard_compact(per_core):
    """(8, 128, CH*PF) -> (192,192,192)."""
    out3 = np.empty((L, L, L), np.float32)
    s0, s1, s2 = out3.strides
    w = np.lib.stride_tricks.as_strided(
        out3,
        shape=(NCORES, KH, JB, CH, JW, KW),
        strides=(CH * s0, KW * s2, JW * s1, s0, s1, s2),
    )
    w[:] = per_core.reshape(NCORES, KH, JB, CH, JW, KW)
    return out3


def _is_const(a):
    a = np.asarray(a)
    return bool(a.size) and bool(np.all(a == a.flat[0]))


def _rolls_ok(nn_idx_1, nn_idx_2, nn_idy_1, nn_idy_2, nn_idz_1, nn_idz_2):
    """Spot-check that the index arrays are the periodic roll stencil."""
    rng = np.random.default_rng(12345)
    f = rng.integers(0, N, size=4096)
    i, r = np.divmod(f, L * L)
    j, k = np.divmod(r, L)

    def flat(ii, jj, kk):
        return (ii % L) * L * L + (jj % L) * L + (kk % L)

    checks = [
        (nn_idx_1, flat(i - 1, j, k)), (nn_idx_2, flat(i + 1, j, k)),
        (nn_idy_1, flat(i, j - 1, k)), (nn_idy_2, flat(i, j + 1, k)),
        (nn_idz_1, flat(i, j, k - 1)), (nn_idz_2, flat(i, j, k + 1)),
    ]
    for arr, want in checks:
        if not np.array_equal(np.asarray(arr)[f], want):
            return False
    return True


def _numpy_fallback(y, J, anisotropy, gamma, h_dis_x, h_dis_y, beta,
                    e_disorder, idx):
    """Exact reference math in numpy (used only if structure checks fail)."""
    x, p = y[:N], y[N:]

    def stencil(v):
        return J * (v[idx[0]] + v[idx[1]] + v[idx[2]] + v[idx[3]]
                    + anisotropy * (v[idx[4]] + v[idx[5]]))

    xL = stencil(x)
    yL = stencil(p)
    r2 = x * x + p * p
    cross = xL * p - yL * x
    dx = gamma * p * cross + e_disorder * p - yL + h_dis_y + beta * r2 * p
    dp = -gamma * x * cross - e_disorder * x + xL - h_dis_x - beta * r2 * x
    return np.concatenate([dx, dp]).astype(np.float32)


# -------------------------------------------------------------- device side

def _build_nc():
    from concourse import bacc
    import concourse.mybir as mybir
    from concourse.tile import TileContext

    ActF = mybir.ActivationFunctionType
    f32 = mybir.dt.float32
    f16 = mybir.dt.float16

    nc = bacc.Bacc("TRN2", target_bir_lowering=False, debug=False,
                   enable_asserts=False, num_devices=NCORES)
    x_in = nc.dram_tensor("x_in", [128, FIN], f16, kind="ExternalInput").ap()
    p_in = nc.dram_tensor("p_in", [128, FIN], f16, kind="ExternalInput").ap()
    # packed per-tile coefficients: [e_disorder | h_dis_x | h_dis_y]
    cf_in = nc.dram_tensor("cf_in", [128, NT, 3, TF], f16, kind="ExternalInput").ap()
    # stencil weights: [0] = J*I, [1] = J*anis*I
    w_in = nc.dram_tensor("w_in", [128, 2, 128], f16, kind="ExternalInput").ap()
    cst_in = nc.dram_tensor("cst_in", [128, 8], f32, kind="ExternalInput").ap()
    dx_out = nc.dram_tensor("dx_out", [128, FOUT], f16, kind="ExternalOutput").ap()
    dp_out = nc.dram_tensor("dp_out", [128, FOUT], f16, kind="ExternalOutput").ap()

    TPL = (T + 2) * PLF          # planes held per tile (incl. i halo)

    with TileContext(nc) as tc:
        with (
            tc.tile_pool(name="persist", bufs=1) as pers,
            tc.tile_pool(name="state", bufs=2) as sp,
            tc.tile_pool(name="coef", bufs=2) as cp,
            tc.tile_pool(name="lap", bufs=2) as lp,
            tc.tile_pool(name="scratch", bufs=2) as tp_,
            tc.tile_pool(name="outs", bufs=2) as op_,
            tc.psum_pool(name="ps", bufs=2) as pp,
        ):
            cst = pers.tile([128, 8], f32, name="cst")
            SQB = cst[:, 0:1]    # sqrt(beta)
            GAM = cst[:, 1:2]    # gamma
            w = pers.tile([128, 2, 128], f16, name="w")
            W1 = w[:, 0]
            W2 = w[:, 1]

            for t in range(NT):
                i0 = t * T
                f0 = i0 * PF

                xt = sp.tile([128, TPL], f16, tag="xt", name=f"xt{t}")
                nc.sync.dma_start(xt[:], x_in[:, i0 * PLF:(i0 + T + 2) * PLF])
                pt = sp.tile([128, TPL], f16, tag="pt", name=f"pt{t}")
                nc.sync.dma_start(pt[:], p_in[:, i0 * PLF:(i0 + T + 2) * PLF])
                if t == 0:
                    nc.sync.dma_start(cst[:], cst_in)
                    nc.sync.dma_start(w[:], w_in)
                ct = cp.tile([128, 3, TF], f16, tag="ct", name=f"ct{t}")
                nc.sync.dma_start(ct[:], cf_in[:, t])
                ed, hx, hy = ct[:, 0, :], ct[:, 1, :], ct[:, 2, :]

                def sl(img, di, dj, dk, c0, cn):
                    """Shifted view of padded tile, planes [c0, c0+cn)."""
                    v = img[:].rearrange("q (i j k) -> q i j k",
                                         i=T + 2, j=FJ, k=FK)
                    return v[:, 1 + c0 + di: 1 + c0 + cn + di,
                             1 + dj: 1 + JW + dj, 1 + dk: 1 + KW + dk]

                xc = sl(xt, 0, 0, 0, 0, T)
                pc = sl(pt, 0, 0, 0, 0, T)

                def v4(tile):
                    return tile[:].rearrange("q (i j k) -> q i j k",
                                             i=T, j=JW, k=KW)

                # ---- squares on ACT: sq = beta * field^2
                SQX = tp_.tile([128, TF], f16, tag="SQX", name=f"SQX{t}")
                SQP = tp_.tile([128, TF], f16, tag="SQP", name=f"SQP{t}")
                nc.scalar.activation(v4(SQX), xc, ActF.Square, scale=SQB)
                nc.scalar.activation(v4(SQP), pc, ActF.Square, scale=SQB)

                # ---- stencil on PE: psum[c] = sum of 6 shifted views
                XL = lp.tile([128, TF], f16, tag="XL", name=f"XL{t}")
                YL = lp.tile([128, TF], f16, tag="YL", name=f"YL{t}")
                SHIFTS1 = ((-1, 0, 0), (1, 0, 0), (0, -1, 0), (0, 1, 0))
                SHIFTS2 = ((0, 0, -1), (0, 0, 1))
                for fi, (img, dst) in enumerate(((xt, XL), (pt, YL))):
                    for r in range(T // 4):
                        ps = pp.tile([128, 4, 512], f32, tag="ps",
                                     name=f"ps_{t}_{fi}_{r}")
                        for b in range(4):
                            c = r * 4 + b
                            po = ps[:, b, :PF]
                            for si, (di, dj, dk) in enumerate(SHIFTS1):
                                nc.tensor.matmul(
                                    po, W1, sl(img, di, dj, dk, c, 1),
                                    start=(si == 0), stop=False)
                            for si, (di, dj, dk) in enumerate(SHIFTS2):
                                nc.tensor.matmul(
                                    po, W2, sl(img, di, dj, dk, c, 1),
                                    start=False, stop=(si == 1))
                        nc.scalar.activation(
                            dst[:, r * 4 * PF:(r + 1) * 4 * PF].rearrange(
                                "q (b s) -> q b s", b=4),
                            ps[:, :, :PF], ActF.Copy)

                # ---- DVE chain (12 two-source ops)
                S1 = tp_.tile([128, TF], f16, tag="S1", name=f"S1_{t}")
                S2 = tp_.tile([128, TF], f16, tag="S2", name=f"S2_{t}")
                U1 = tp_.tile([128, TF], f16, tag="U1", name=f"U1_{t}")
                U2 = tp_.tile([128, TF], f16, tag="U2", name=f"U2_{t}")

                nc.vector.tensor_mul(v4(S1), v4(XL), pc)             # m1
                nc.vector.tensor_mul(v4(S2), v4(YL), xc)             # m2
                nc.vector.tensor_sub(S1[:], S1[:], S2[:])            # cross
                # cg = gamma * cross on ACT (DVE does u1/u2 meanwhile)
                nc.scalar.activation(S2[:], S1[:], ActF.Copy, scale=GAM)
                nc.vector.tensor_sub(U1[:], hy, YL[:])               # u1
                nc.vector.tensor_sub(U2[:], XL[:], hx)               # u2
                nc.vector.tensor_add(S2[:], S2[:], ed)               # + ed
                nc.vector.tensor_add(S2[:], S2[:], SQX[:])           # + b x^2
                nc.vector.tensor_add(S2[:], S2[:], SQP[:])           # s2
                dxo = op_.tile([128, TF], f16, tag="dxo", name=f"dxo{t}")
                dpo = op_.tile([128, TF], f16, tag="dpo", name=f"dpo{t}")
                nc.vector.tensor_mul(v4(SQX), pc, v4(S2))            # t1
                nc.vector.tensor_add(dxo[:], SQX[:], U1[:])          # dx
                nc.sync.dma_start(dx_out[:, f0:f0 + TF], dxo[:])
                nc.vector.tensor_mul(v4(SQP), xc, v4(S2))            # t2
                nc.vector.tensor_sub(dpo[:], U2[:], SQP[:])          # dp
                nc.sync.dma_start(dp_out[:, f0:f0 + TF], dpo[:])

    nc.compile()
    return nc


def _get_nc():
    if "nc" not in _STATE:
        _STATE["nc"] = _build_nc()
    return _STATE["nc"]


def _run(in_maps, trace=False, trace_cores=None):
    from concourse.bass_utils import run_bass_kernel_spmd
    if trace:
        # the agent image's antenv lacks axon_hooks; wire the NTFF hook
        import sys as _sys
        import types as _types
        if "antenv.axon_hooks" not in _sys.modules:
            try:
                import trn_agent_boot.trn_boot as _tb
                _hook = _tb._ntff_profile_via_ctypes('/opt/axon/libaxon_pjrt.so')
                _mod = _types.ModuleType("antenv.axon_hooks")
                _mod.get_axon_ntff_profile_hook = lambda: _hook
                _sys.modules["antenv.axon_hooks"] = _mod
            except Exception:
                pass
    return run_bass_kernel_spmd(
        _get_nc(), in_maps, core_ids=list(range(NCORES)),
        trace=trace, trace_cores=trace_cores,
    )


def prepare_in_maps(y, anis_v, gamma_v, beta_v, j_v, h_dis_x, h_dis_y,
                    e_disorder):
    """Host-side sharding: build the 8 per-core input maps (fp16)."""
    x3 = np.asarray(y[:N], np.float32).reshape(L, L, L)
    p3 = np.asarray(y[N:], np.float32).reshape(L, L, L)
    xs = _shard_halo(np.pad(x3, 1, mode="wrap").astype(np.float16))
    ps = _shard_halo(np.pad(p3, 1, mode="wrap").astype(np.float16))
    eds = _shard_compact(
        np.asarray(e_disorder, np.float16).reshape(L, L, L))
    hxs = _shard_compact(
        np.asarray(h_dis_x, np.float16).reshape(L, L, L))
    hys = _shard_compact(
        np.asarray(h_dis_y, np.float16).reshape(L, L, L))
    # pack per-tile coefficient blocks: [NT, 3, TF]
    cf = np.stack([eds.reshape(NCORES, 128, NT, TF),
                   hxs.reshape(NCORES, 128, NT, TF),
                   hys.reshape(NCORES, 128, NT, TF)], axis=3)
    cf = np.ascontiguousarray(cf)          # (8, 128, NT, 3, TF)
    w = np.zeros((128, 2, 128), np.float16)
    d = np.arange(128)
    w[:, 0][d, d] = np.float16(j_v)
    w[:, 1][d, d] = np.float16(j_v * anis_v)
    cst = np.zeros((128, 8), np.float32)
    cst[:, 0] = np.sqrt(beta_v)
    cst[:, 1] = gamma_v
    return [
        {"x_in": xs[c], "p_in": ps[c], "cf_in": cf[c], "w_in": w,
         "cst_in": cst}
        for c in range(NCORES)
    ]


def assemble_output(results):
    """Per-core device outputs -> full (2N,) float32 array."""
    dxs = np.stack([results[c]["dx_out"] for c in range(NCORES)]
                   ).astype(np.float32)
    dps = np.stack([results[c]["dp_out"] for c in range(NCORES)]
                   ).astype(np.float32)
    dx3 = _unshard_compact(dxs)
    dp3 = _unshard_compact(dps)
    return np.concatenate([dx3.reshape(-1), dp3.reshape(-1)])


def kernel(t, y, J, anisotropy, gamma, h_dis_x, h_dis_y, beta, e_disorder,
           nn_idx_1, nn_idx_2, nn_idy_1, nn_idy_2, nn_idz_1, nn_idz_2):
    y = np.asarray(y, np.float32)
    J = np.asarray(J, np.float32)
    anisotropy = np.asarray(anisotropy, np.float32)
    gamma = np.asarray(gamma, np.float32)
    beta = np.asarray(beta, np.float32)
    h_dis_x = np.asarray(h_dis_x, np.float32)
    h_dis_y = np.asarray(h_dis_y, np.float32)
    e_disorder = np.asarray(e_disorder, np.float32)

    ok = (y.shape == (2 * N,)
          and _is_const(J) and _is_const(anisotropy)
          and _is_const(gamma) and _is_const(beta) and float(beta.flat[0]) >= 0
          and _rolls_ok(nn_idx_1, nn_idx_2, nn_idy_1, nn_idy_2,
                        nn_idz_1, nn_idz_2))
    if not ok:
        idx = [np.asarray(a) for a in (nn_idx_1, nn_idx_2, nn_idy_1,
                                       nn_idy_2, nn_idz_1, nn_idz_2)]
        return _numpy_fallback(y, J, anisotropy, gamma, h_dis_x, h_dis_y,
                               beta, e_disorder, idx)

    in_maps = prepare_in_maps(
        y, float(anisotropy.flat[0]), float(gamma.flat[0]),
        float(beta.flat[0]), float(J.flat[0]), h_dis_x, h_dis_y, e_disorder)
    res = _run(in_maps, trace=False)
    return assemble_output(res.results)


# revision 6
# speedup vs baseline: 1.8875x; 1.0156x over previous
"""Bass/Trainium2 kernel for the DGPE relaxation RHS on a 192^3 periodic lattice.

The nn_id* inputs are the deterministic 6-neighbor roll indices of the
lattice, so the gathers are implemented as stencil shifts.  The lattice is
sharded along axis 0 across 8 NeuronCores (24 planes + 2 halo planes each,
sliced host-side).  Within a core, partition = (k-block, j-block) = 8 x 16,
each partition holding a (24 x 12 x 24) sub-brick stored with j/k halo
strips so every neighbor access is a plain access-pattern offset.

v2: all device data is fp16 (DVE 2x mode, half DMA traffic).  The full
6-neighbor stencil runs on the idle PE as identity-weight matmuls over
shifted views accumulating in PSUM (J and J*anisotropy on the diagonals),
the squares / psum->sbuf copies / gamma scale run on ACT, leaving DVE
only the 12 irreducible two-source elementwise ops.
"""

import numpy as np

L = 192
N = L ** 3
NCORES = 8
CH = L // NCORES            # 24 planes (axis 0) per core
KH, JB = 8, 16              # partition grid: p = kh*JB + jb
JW = L // JB                # 12 j's per partition
KW = L // KH                # 24 k's per partition
IH = CH + 2                 # 26 planes incl. axis-0 halo
FJ = JW + 2                 # 14 incl. j halo strips
FK = KW + 2                 # 26 incl. k halo strips
PLF = FJ * FK               # padded plane free size (364)
FIN = IH * PLF
PF = JW * KW                # compact plane free size (288)
FOUT = CH * PF
T = 4                       # planes per compute tile
NT = CH // T
TF = T * PF

_STATE = {}


# ---------------------------------------------------------------- host side

def _shard_halo(v3pad):
    """(194,194,194) wrap-padded -> (8, 128, FIN) per-core images."""
    s0, s1, s2 = v3pad.strides
    v = np.lib.stride_tricks.as_strided(
        v3pad,
        shape=(NCORES, KH, JB, IH, FJ, FK),
        strides=(CH * s0, KW * s2, JW * s1, s0, s1, s2),
    )
    return np.ascontiguousarray(v).reshape(NCORES, 128, FIN)


def _shard_compact(v3):
    """(192,192,192) -> (8, 128, CH, PF) per-core compact images."""
    s0, s1, s2 = v3.strides
    v = np.lib.stride_tricks.as_strided(
        v3,
        shape=(NCORES, KH, JB, CH, JW, KW),
        strides=(CH * s0, KW * s2, JW * s1, s0, s1, s2),
    )
    return np.ascontiguousarray(v).reshape(NCORES, 128, CH, PF)


def _unshard_compact(per_core):
    """(8, 128, CH*PF) -> (192,192,192)."""
    out3 = np.empty((L, L, L), np.float32)
    s0, s1, s2 = out3.strides
    w = np.lib.stride_tricks.as_strided(
        out3,
        shape=(NCORES, KH, JB, CH, JW, KW),
        strides=(CH * s0, KW * s2, JW * s1, s0, s1, s2),
    )
    w[:] = per_core.reshape(NCORES, KH, JB, CH, JW, KW)
    return out3


def _is_const(a):
    a = np.asarray(a)
    return bool(a.size) and bool(np.all(a == a.flat[0]))


def _rolls_ok(nn_idx_1, nn_idx_2, nn_idy_1, nn_idy_2, nn_idz_1, nn_idz_2):
    """Spot-check that the index arrays are the periodic roll stencil."""
    rng = np.random.default_rng(12345)
    f = rng.integers(0, N, size=4096)
    i, r = np.divmod(f, L * L)
    j, k = np.divmod(r, L)

    def flat(ii, jj, kk):
        return (ii % L) * L * L + (jj % L) * L + (kk % L)

    checks = [
        (nn_idx_1, flat(i - 1, j, k)), (nn_idx_2, flat(i + 1, j, k)),
        (nn_idy_1, flat(i, j - 1, k)), (nn_idy_2, flat(i, j + 1, k)),
        (nn_idz_1, flat(i, j, k - 1)), (nn_idz_2, flat(i, j, k + 1)),
    ]
    for arr, want in checks:
        if not np.array_equal(np.asarray(arr)[f], want):
            return False
    return True


def _numpy_fallback(y, J, anisotropy, gamma, h_dis_x, h_dis_y, beta,
                    e_disorder, idx):
    """Exact reference math in numpy (used only if structure checks fail)."""
    x, p = y[:N], y[N:]

    def stencil(v):
        return J * (v[idx[0]] + v[idx[1]] + v[idx[2]] + v[idx[3]]
                    + anisotropy * (v[idx[4]] + v[idx[5]]))

    xL = stencil(x)
    yL = stencil(p)
    r2 = x * x + p * p
    cross = xL * p - yL * x
    dx = gamma * p * cross + e_disorder * p - yL + h_dis_y + beta * r2 * p
    dp = -gamma * x * cross - e_disorder * x + xL - h_dis_x - beta * r2 * x
    return np.concatenate([dx, dp]).astype(np.float32)


# -------------------------------------------------------------- device side

def _build_nc():
    from concourse import bacc
    import concourse.mybir as mybir
    from concourse.tile import TileContext

    ActF = mybir.ActivationFunctionType
    f32 = mybir.dt.float32
    f16 = mybir.dt.float16

    nc = bacc.Bacc("TRN2", target_bir_lowering=False, debug=False,
                   enable_asserts=False, num_devices=NCORES)
    x_in = nc.dram_tensor("x_in", [128, FIN], f16, kind="ExternalInput").ap()
    p_in = nc.dram_tensor("p_in", [128, FIN], f16, kind="ExternalInput").ap()
    # packed per-tile coefficients: [e_disorder | h_dis_x | h_dis_y]
    cf_in = nc.dram_tensor("cf_in", [128, NT, 3, TF], f16, kind="ExternalInput").ap()
    # stencil weights: [0] = J*I, [1] = J*anis*I
    w_in = nc.dram_tensor("w_in", [128, 2, 128], f16, kind="ExternalInput").ap()
    cst_in = nc.dram_tensor("cst_in", [128, 8], f32, kind="ExternalInput").ap()
    dx_out = nc.dram_tensor("dx_out", [128, FOUT], f16, kind="ExternalOutput").ap()
    dp_out = nc.dram_tensor("dp_out", [128, FOUT], f16, kind="ExternalOutput").ap()

    TPL = (T + 2) * PLF          # planes held per tile (incl. i halo)

    with TileContext(nc) as tc:
        with (
            tc.tile_pool(name="persist", bufs=1) as pers,
            tc.tile_pool(name="state", bufs=2) as sp,
            tc.tile_pool(name="coef", bufs=2) as cp,
            tc.tile_pool(name="lap", bufs=2) as lp,
            tc.tile_pool(name="scratch", bufs=2) as tp_,
            tc.tile_pool(name="outs", bufs=2) as op_,
            tc.psum_pool(name="ps", bufs=2) as pp,
        ):
            cst = pers.tile([128, 8], f32, name="cst")
            SQB = cst[:, 0:1]    # sqrt(beta)
            GAM = cst[:, 1:2]    # gamma
            w = pers.tile([128, 2, 128], f16, name="w")
            W1 = w[:, 0]
            W2 = w[:, 1]
            nc.sync.dma_start(cst[:], cst_in)
            nc.sync.dma_start(w[:], w_in)

            # PE pstate warmup: ~24 small matmuls while the first state
            # tiles stream in (ramps the PE clock toward 2.4 GHz)
            wps = pp.tile([128, 4, 512], f32, tag="ps", name="warm")
            for i in range(24):
                nc.tensor.matmul(wps[:, i % 4, :128], W1, W1,
                                 start=True, stop=True)

            for t in range(NT):
                i0 = t * T
                f0 = i0 * PF

                xt = sp.tile([128, TPL], f16, tag="xt", name=f"xt{t}")
                nc.sync.dma_start(xt[:], x_in[:, i0 * PLF:(i0 + T + 2) * PLF])
                pt = sp.tile([128, TPL], f16, tag="pt", name=f"pt{t}")
                nc.sync.dma_start(pt[:], p_in[:, i0 * PLF:(i0 + T + 2) * PLF])
                ct = cp.tile([128, 3, TF], f16, tag="ct", name=f"ct{t}")
                nc.sync.dma_start(ct[:], cf_in[:, t])
                ed, hx, hy = ct[:, 0, :], ct[:, 1, :], ct[:, 2, :]

                def sl(img, di, dj, dk, c0, cn):
                    """Shifted view of padded tile, planes [c0, c0+cn)."""
                    v = img[:].rearrange("q (i j k) -> q i j k",
                                         i=T + 2, j=FJ, k=FK)
                    return v[:, 1 + c0 + di: 1 + c0 + cn + di,
                             1 + dj: 1 + JW + dj, 1 + dk: 1 + KW + dk]

                xc = sl(xt, 0, 0, 0, 0, T)
                pc = sl(pt, 0, 0, 0, 0, T)

                def v4(tile):
                    return tile[:].rearrange("q (i j k) -> q i j k",
                                             i=T, j=JW, k=KW)

                # ---- squares on ACT: sq = beta * field^2
                SQX = tp_.tile([128, TF], f16, tag="SQX", name=f"SQX{t}")
                SQP = tp_.tile([128, TF], f16, tag="SQP", name=f"SQP{t}")
                nc.scalar.activation(v4(SQX), xc, ActF.Square, scale=SQB)
                nc.scalar.activation(v4(SQP), pc, ActF.Square, scale=SQB)

                # ---- stencil on PE: psum[c] = sum of 6 shifted views
                XL = lp.tile([128, TF], f16, tag="XL", name=f"XL{t}")
                YL = lp.tile([128, TF], f16, tag="YL", name=f"YL{t}")
                SHIFTS1 = ((-1, 0, 0), (1, 0, 0), (0, -1, 0), (0, 1, 0))
                SHIFTS2 = ((0, 0, -1), (0, 0, 1))
                for fi, (img, dst) in enumerate(((xt, XL), (pt, YL))):
                    ps = pp.tile([128, 4, 512], f32, tag="ps",
                                 name=f"ps_{t}_{fi}")
                    for c in range(T):
                        po = ps[:, c, :PF]
                        for si, (di, dj, dk) in enumerate(SHIFTS1):
                            nc.tensor.matmul(
                                po, W1, sl(img, di, dj, dk, c, 1),
                                start=(si == 0), stop=False)
                        for si, (di, dj, dk) in enumerate(SHIFTS2):
                            nc.tensor.matmul(
                                po, W2, sl(img, di, dj, dk, c, 1),
                                start=False, stop=(si == 1))
                    nc.scalar.activation(
                        dst[:].rearrange("q (b s) -> q b s", b=4),
                        ps[:, :, :PF], ActF.Copy)

                # ---- DVE chain (12 two-source ops + gamma 1-src)
                S1 = tp_.tile([128, TF], f16, tag="S1", name=f"S1_{t}")
                S2 = tp_.tile([128, TF], f16, tag="S2", name=f"S2_{t}")
                U1 = tp_.tile([128, TF], f16, tag="U1", name=f"U1_{t}")
                U2 = tp_.tile([128, TF], f16, tag="U2", name=f"U2_{t}")

                nc.vector.tensor_mul(v4(S1), v4(XL), pc)             # m1
                nc.vector.tensor_mul(v4(S2), v4(YL), xc)             # m2
                nc.vector.tensor_sub(S1[:], S1[:], S2[:])            # cross
                nc.vector.tensor_scalar_mul(S1[:], S1[:], GAM)       # * gamma
                nc.vector.tensor_sub(U1[:], hy, YL[:])               # u1
                nc.vector.tensor_sub(U2[:], XL[:], hx)               # u2
                nc.vector.tensor_add(S1[:], S1[:], ed)               # + ed
                nc.vector.tensor_add(S1[:], S1[:], SQX[:])           # + b x^2
                nc.vector.tensor_add(S1[:], S1[:], SQP[:])           # s2
                dxo = op_.tile([128, TF], f16, tag="dxo", name=f"dxo{t}")
                dpo = op_.tile([128, TF], f16, tag="dpo", name=f"dpo{t}")
                nc.vector.tensor_mul(v4(S2), pc, v4(S1))             # t1
                nc.vector.tensor_add(dxo[:], S2[:], U1[:])           # dx
                nc.sync.dma_start(dx_out[:, f0:f0 + TF], dxo[:])
                nc.vector.tensor_mul(v4(S2), xc, v4(S1))             # t2
                nc.vector.tensor_sub(dpo[:], U2[:], S2[:])           # dp
                nc.sync.dma_start(dp_out[:, f0:f0 + TF], dpo[:])

    nc.compile()
    return nc


def _get_nc():
    if "nc" not in _STATE:
        _STATE["nc"] = _build_nc()
    return _STATE["nc"]


def _run(in_maps, trace=False, trace_cores=None):
    from concourse.bass_utils import run_bass_kernel_spmd
    if trace:
        # the agent image's antenv lacks axon_hooks; wire the NTFF hook
        import sys as _sys
        import types as _types
        if "antenv.axon_hooks" not in _sys.modules:
            try:
                import trn_agent_boot.trn_boot as _tb
                _hook = _tb._ntff_profile_via_ctypes('/opt/axon/libaxon_pjrt.so')
                _mod = _types.ModuleType("antenv.axon_hooks")
                _mod.get_axon_ntff_profile_hook = lambda: _hook
                _sys.modules["antenv.axon_hooks"] = _mod
            except Exception:
                pass
    return run_bass_kernel_spmd(
        _get_nc(), in_maps, core_ids=list(range(NCORES)),
        trace=trace, trace_cores=trace_cores,
    )


def prepare_in_maps(y, anis_v, gamma_v, beta_v, j_v, h_dis_x, h_dis_y,
                    e_disorder):
    """Host-side sharding: build the 8 per-core input maps (fp16)."""
    x3 = np.asarray(y[:N], np.float32).reshape(L, L, L)
    p3 = np.asarray(y[N:], np.float32).reshape(L, L, L)
    xs = _shard_halo(np.pad(x3, 1, mode="wrap").astype(np.float16))
    ps = _shard_halo(np.pad(p3, 1, mode="wrap").astype(np.float16))
    eds = _shard_compact(
        np.asarray(e_disorder, np.float16).reshape(L, L, L))
    hxs = _shard_compact(
        np.asarray(h_dis_x, np.float16).reshape(L, L, L))
    hys = _shard_compact(
        np.asarray(h_dis_y, np.float16).reshape(L, L, L))
    # pack per-tile coefficient blocks: [NT, 3, TF]
    cf = np.stack([eds.reshape(NCORES, 128, NT, TF),
                   hxs.reshape(NCORES, 128, NT, TF),
                   hys.reshape(NCORES, 128, NT, TF)], axis=3)
    cf = np.ascontiguousarray(cf)          # (8, 128, NT, 3, TF)
    w = np.zeros((128, 2, 128), np.float16)
    d = np.arange(128)
    w[:, 0][d, d] = np.float16(j_v)
    w[:, 1][d, d] = np.float16(j_v * anis_v)
    cst = np.zeros((128, 8), np.float32)
    cst[:, 0] = np.sqrt(beta_v)
    cst[:, 1] = gamma_v
    return [
        {"x_in": xs[c], "p_in": ps[c], "cf_in": cf[c], "w_in": w,
         "cst_in": cst}
        for c in range(NCORES)
    ]


def assemble_output(results):
    """Per-core device outputs -> full (2N,) float32 array."""
    dxs = np.stack([results[c]["dx_out"] for c in range(NCORES)]
                   ).astype(np.float32)
    dps = np.stack([results[c]["dp_out"] for c in range(NCORES)]
                   ).astype(np.float32)
    dx3 = _unshard_compact(dxs)
    dp3 = _unshard_compact(dps)
    return np.concatenate([dx3.reshape(-1), dp3.reshape(-1)])


def kernel(t, y, J, anisotropy, gamma, h_dis_x, h_dis_y, beta, e_disorder,
           nn_idx_1, nn_idx_2, nn_idy_1, nn_idy_2, nn_idz_1, nn_idz_2):
    y = np.asarray(y, np.float32)
    J = np.asarray(J, np.float32)
    anisotropy = np.asarray(anisotropy, np.float32)
    gamma = np.asarray(gamma, np.float32)
    beta = np.asarray(beta, np.float32)
    h_dis_x = np.asarray(h_dis_x, np.float32)
    h_dis_y = np.asarray(h_dis_y, np.float32)
    e_disorder = np.asarray(e_disorder, np.float32)

    ok = (y.shape == (2 * N,)
          and _is_const(J) and _is_const(anisotropy)
          and _is_const(gamma) and _is_const(beta) and float(beta.flat[0]) >= 0
          and _rolls_ok(nn_idx_1, nn_idx_2, nn_idy_1, nn_idy_2,
                        nn_idz_1, nn_idz_2))
    if not ok:
        idx = [np.asarray(a) for a in (nn_idx_1, nn_idx_2, nn_idy_1,
                                       nn_idy_2, nn_idz_1, nn_idz_2)]
        return _numpy_fallback(y, J, anisotropy, gamma, h_dis_x, h_dis_y,
                               beta, e_disorder, idx)

    in_maps = prepare_in_maps(
        y, float(anisotropy.flat[0]), float(gamma.flat[0]),
        float(beta.flat[0]), float(J.flat[0]), h_dis_x, h_dis_y, e_disorder)
    res = _run(in_maps, trace=False)
    return assemble_output(res.results)


# revision 8
# speedup vs baseline: 1.9208x; 1.0176x over previous
"""Bass/Trainium2 kernel for the DGPE relaxation RHS on a 192^3 periodic lattice.

The nn_id* inputs are the deterministic 6-neighbor roll indices of the
lattice, so the gathers are implemented as stencil shifts.  The lattice is
sharded along axis 0 across 8 NeuronCores (24 planes + 2 halo planes each,
sliced host-side).  Within a core, partition = (k-block, j-block) = 8 x 16,
each partition holding a (24 x 12 x 24) sub-brick stored with j/k halo
strips so every neighbor access is a plain access-pattern offset.

v2: all device data is fp16 (DVE 2x mode, half DMA traffic).  The full
6-neighbor stencil runs on the idle PE as identity-weight matmuls over
shifted views accumulating in PSUM (J and J*anisotropy on the diagonals),
the squares / psum->sbuf copies / gamma scale run on ACT, leaving DVE
only the 12 irreducible two-source elementwise ops.
"""

import numpy as np

L = 192
N = L ** 3
NCORES = 8
CH = L // NCORES            # 24 planes (axis 0) per core
KH, JB = 8, 16              # partition grid: p = kh*JB + jb
JW = L // JB                # 12 j's per partition
KW = L // KH                # 24 k's per partition
IH = CH + 2                 # 26 planes incl. axis-0 halo
FJ = JW + 2                 # 14 incl. j halo strips
FK = KW + 2                 # 26 incl. k halo strips
PLF = FJ * FK               # padded plane free size (364)
FIN = IH * PLF
PF = JW * KW                # compact plane free size (288)
FOUT = CH * PF
T = 8                       # planes per compute tile
NT = CH // T
TF = T * PF

_STATE = {}


# ---------------------------------------------------------------- host side

def _shard_halo(v3pad):
    """(194,194,194) wrap-padded -> (8, 128, FIN) per-core images."""
    s0, s1, s2 = v3pad.strides
    v = np.lib.stride_tricks.as_strided(
        v3pad,
        shape=(NCORES, KH, JB, IH, FJ, FK),
        strides=(CH * s0, KW * s2, JW * s1, s0, s1, s2),
    )
    return np.ascontiguousarray(v).reshape(NCORES, 128, FIN)


def _shard_compact(v3):
    """(192,192,192) -> (8, 128, CH, PF) per-core compact images."""
    s0, s1, s2 = v3.strides
    v = np.lib.stride_tricks.as_strided(
        v3,
        shape=(NCORES, KH, JB, CH, JW, KW),
        strides=(CH * s0, KW * s2, JW * s1, s0, s1, s2),
    )
    return np.ascontiguousarray(v).reshape(NCORES, 128, CH, PF)


def _unshard_compact(per_core):
    """(8, 128, CH*PF) -> (192,192,192)."""
    out3 = np.empty((L, L, L), np.float32)
    s0, s1, s2 = out3.strides
    w = np.lib.stride_tricks.as_strided(
        out3,
        shape=(NCORES, KH, JB, CH, JW, KW),
        strides=(CH * s0, KW * s2, JW * s1, s0, s1, s2),
    )
    w[:] = per_core.reshape(NCORES, KH, JB, CH, JW, KW)
    return out3


def _is_const(a):
    a = np.asarray(a)
    return bool(a.size) and bool(np.all(a == a.flat[0]))


def _rolls_ok(nn_idx_1, nn_idx_2, nn_idy_1, nn_idy_2, nn_idz_1, nn_idz_2):
    """Spot-check that the index arrays are the periodic roll stencil."""
    rng = np.random.default_rng(12345)
    f = rng.integers(0, N, size=4096)
    i, r = np.divmod(f, L * L)
    j, k = np.divmod(r, L)

    def flat(ii, jj, kk):
        return (ii % L) * L * L + (jj % L) * L + (kk % L)

    checks = [
        (nn_idx_1, flat(i - 1, j, k)), (nn_idx_2, flat(i + 1, j, k)),
        (nn_idy_1, flat(i, j - 1, k)), (nn_idy_2, flat(i, j + 1, k)),
        (nn_idz_1, flat(i, j, k - 1)), (nn_idz_2, flat(i, j, k + 1)),
    ]
    for arr, want in checks:
        if not np.array_equal(np.asarray(arr)[f], want):
            return False
    return True


def _numpy_fallback(y, J, anisotropy, gamma, h_dis_x, h_dis_y, beta,
                    e_disorder, idx):
    """Exact reference math in numpy (used only if structure checks fail)."""
    x, p = y[:N], y[N:]

    def stencil(v):
        return J * (v[idx[0]] + v[idx[1]] + v[idx[2]] + v[idx[3]]
                    + anisotropy * (v[idx[4]] + v[idx[5]]))

    xL = stencil(x)
    yL = stencil(p)
    r2 = x * x + p * p
    cross = xL * p - yL * x
    dx = gamma * p * cross + e_disorder * p - yL + h_dis_y + beta * r2 * p
    dp = -gamma * x * cross - e_disorder * x + xL - h_dis_x - beta * r2 * x
    return np.concatenate([dx, dp]).astype(np.float32)


# -------------------------------------------------------------- device side

def _build_nc():
    from concourse import bacc
    import concourse.mybir as mybir
    from concourse.tile import TileContext

    ActF = mybir.ActivationFunctionType
    f32 = mybir.dt.float32
    f16 = mybir.dt.float16

    nc = bacc.Bacc("TRN2", target_bir_lowering=False, debug=False,
                   enable_asserts=False, num_devices=NCORES)
    x_in = nc.dram_tensor("x_in", [128, FIN], f16, kind="ExternalInput").ap()
    p_in = nc.dram_tensor("p_in", [128, FIN], f16, kind="ExternalInput").ap()
    # packed per-tile coefficients: [e_disorder | h_dis_x | h_dis_y]
    cf_in = nc.dram_tensor("cf_in", [128, NT, 3, TF], f16, kind="ExternalInput").ap()
    # stencil weights: [0] = J*I, [1] = J*anis*I
    w_in = nc.dram_tensor("w_in", [128, 2, 128], f16, kind="ExternalInput").ap()
    cst_in = nc.dram_tensor("cst_in", [128, 8], f32, kind="ExternalInput").ap()
    dx_out = nc.dram_tensor("dx_out", [128, FOUT], f16, kind="ExternalOutput").ap()
    dp_out = nc.dram_tensor("dp_out", [128, FOUT], f16, kind="ExternalOutput").ap()

    TPL = (T + 2) * PLF          # planes held per tile (incl. i halo)

    with TileContext(nc) as tc:
        with (
            tc.tile_pool(name="persist", bufs=1) as pers,
            tc.tile_pool(name="state", bufs=2) as sp,
            tc.tile_pool(name="coef", bufs=2) as cp,
            tc.tile_pool(name="lap", bufs=2) as lp,
            tc.tile_pool(name="scratch", bufs=2) as tp_,
            tc.tile_pool(name="outs", bufs=2) as op_,
            tc.psum_pool(name="ps", bufs=2) as pp,
        ):
            cst = pers.tile([128, 8], f32, name="cst")
            SQB = cst[:, 0:1]    # sqrt(beta)
            GAM = cst[:, 1:2]    # gamma
            w = pers.tile([128, 2, 128], f16, name="w")
            W1 = w[:, 0]
            W2 = w[:, 1]
            nc.sync.dma_start(cst[:], cst_in)
            nc.sync.dma_start(w[:], w_in)

            # PE pstate warmup: ~24 small matmuls while the first state
            # tiles stream in (ramps the PE clock toward 2.4 GHz)
            wps = pp.tile([128, 4, 512], f32, tag="ps", name="warm")
            for i in range(24):
                nc.tensor.matmul(wps[:, i % 4, :128], W1, W1,
                                 start=True, stop=True)

            for t in range(NT):
                i0 = t * T
                f0 = i0 * PF

                xt = sp.tile([128, TPL], f16, tag="xt", name=f"xt{t}")
                nc.sync.dma_start(xt[:], x_in[:, i0 * PLF:(i0 + T + 2) * PLF])
                pt = sp.tile([128, TPL], f16, tag="pt", name=f"pt{t}")
                nc.sync.dma_start(pt[:], p_in[:, i0 * PLF:(i0 + T + 2) * PLF])
                ct = cp.tile([128, 3, TF], f16, tag="ct", name=f"ct{t}")
                nc.sync.dma_start(ct[:], cf_in[:, t])
                ed, hx, hy = ct[:, 0, :], ct[:, 1, :], ct[:, 2, :]

                def sl(img, di, dj, dk, c0, cn):
                    """Shifted view of padded tile, planes [c0, c0+cn)."""
                    v = img[:].rearrange("q (i j k) -> q i j k",
                                         i=T + 2, j=FJ, k=FK)
                    return v[:, 1 + c0 + di: 1 + c0 + cn + di,
                             1 + dj: 1 + JW + dj, 1 + dk: 1 + KW + dk]

                xc = sl(xt, 0, 0, 0, 0, T)
                pc = sl(pt, 0, 0, 0, 0, T)

                def v4(tile):
                    return tile[:].rearrange("q (i j k) -> q i j k",
                                             i=T, j=JW, k=KW)

                # ---- stencil on PE: psum[c] = sum of 6 shifted views
                XL = lp.tile([128, TF], f16, tag="XL", name=f"XL{t}")
                YL = lp.tile([128, TF], f16, tag="YL", name=f"YL{t}")
                SHIFTS1 = ((-1, 0, 0), (1, 0, 0), (0, -1, 0), (0, 1, 0))
                SHIFTS2 = ((0, 0, -1), (0, 0, 1))
                for fi, (img, dst) in enumerate(((xt, XL), (pt, YL))):
                    for r in range(T // 4):
                        ps = pp.tile([128, 4, 512], f32, tag="ps",
                                     name=f"ps_{t}_{fi}_{r}")
                        for b in range(4):
                            c = r * 4 + b
                            po = ps[:, b, :PF]
                            for si, (di, dj, dk) in enumerate(SHIFTS1):
                                nc.tensor.matmul(
                                    po, W1, sl(img, di, dj, dk, c, 1),
                                    start=(si == 0), stop=False)
                            for si, (di, dj, dk) in enumerate(SHIFTS2):
                                nc.tensor.matmul(
                                    po, W2, sl(img, di, dj, dk, c, 1),
                                    start=False, stop=(si == 1))
                        nc.scalar.activation(
                            dst[:, r * 4 * PF:(r + 1) * 4 * PF].rearrange(
                                "q (b s) -> q b s", b=4),
                            ps[:, :, :PF], ActF.Copy)

                # ---- squares on ACT (after the psum copies in queue order:
                # the DVE chain needs XL/YL first, the squares only later)
                SQX = tp_.tile([128, TF], f16, tag="SQX", name=f"SQX{t}")
                SQP = tp_.tile([128, TF], f16, tag="SQP", name=f"SQP{t}")
                nc.scalar.activation(v4(SQX), xc, ActF.Square, scale=SQB)
                nc.scalar.activation(v4(SQP), pc, ActF.Square, scale=SQB)

                # ---- DVE chain (12 two-source ops + gamma 1-src)
                S1 = tp_.tile([128, TF], f16, tag="S1", name=f"S1_{t}")
                S2 = tp_.tile([128, TF], f16, tag="S2", name=f"S2_{t}")
                U1 = tp_.tile([128, TF], f16, tag="U1", name=f"U1_{t}")
                U2 = tp_.tile([128, TF], f16, tag="U2", name=f"U2_{t}")

                nc.vector.tensor_mul(v4(S1), v4(XL), pc)             # m1
                nc.vector.tensor_mul(v4(S2), v4(YL), xc)             # m2
                nc.vector.tensor_sub(S1[:], S1[:], S2[:])            # cross
                nc.vector.tensor_scalar_mul(S1[:], S1[:], GAM)       # * gamma
                nc.vector.tensor_sub(U1[:], hy, YL[:])               # u1
                nc.vector.tensor_sub(U2[:], XL[:], hx)               # u2
                nc.vector.tensor_add(S1[:], S1[:], ed)               # + ed
                nc.vector.tensor_add(S1[:], S1[:], SQX[:])           # + b x^2
                nc.vector.tensor_add(S1[:], S1[:], SQP[:])           # s2
                dxo = op_.tile([128, TF], f16, tag="dxo", name=f"dxo{t}")
                dpo = op_.tile([128, TF], f16, tag="dpo", name=f"dpo{t}")
                nc.vector.tensor_mul(v4(S2), pc, v4(S1))             # t1
                nc.vector.tensor_add(dxo[:], S2[:], U1[:])           # dx
                nc.sync.dma_start(dx_out[:, f0:f0 + TF], dxo[:])
                nc.vector.tensor_mul(v4(S2), xc, v4(S1))             # t2
                nc.vector.tensor_sub(dpo[:], U2[:], S2[:])           # dp
                nc.sync.dma_start(dp_out[:, f0:f0 + TF], dpo[:])

    nc.compile()
    return nc


def _get_nc():
    if "nc" not in _STATE:
        _STATE["nc"] = _build_nc()
    return _STATE["nc"]


def _run(in_maps, trace=False, trace_cores=None):
    from concourse.bass_utils import run_bass_kernel_spmd
    if trace:
        # the agent image's antenv lacks axon_hooks; wire the NTFF hook
        import sys as _sys
        import types as _types
        if "antenv.axon_hooks" not in _sys.modules:
            try:
                import trn_agent_boot.trn_boot as _tb
                _hook = _tb._ntff_profile_via_ctypes('/opt/axon/libaxon_pjrt.so')
                _mod = _types.ModuleType("antenv.axon_hooks")
                _mod.get_axon_ntff_profile_hook = lambda: _hook
                _sys.modules["antenv.axon_hooks"] = _mod
            except Exception:
                pass
    return run_bass_kernel_spmd(
        _get_nc(), in_maps, core_ids=list(range(NCORES)),
        trace=trace, trace_cores=trace_cores,
    )


def prepare_in_maps(y, anis_v, gamma_v, beta_v, j_v, h_dis_x, h_dis_y,
                    e_disorder):
    """Host-side sharding: build the 8 per-core input maps (fp16)."""
    x3 = np.asarray(y[:N], np.float32).reshape(L, L, L)
    p3 = np.asarray(y[N:], np.float32).reshape(L, L, L)
    xs = _shard_halo(np.pad(x3, 1, mode="wrap").astype(np.float16))
    ps = _shard_halo(np.pad(p3, 1, mode="wrap").astype(np.float16))
    eds = _shard_compact(
        np.asarray(e_disorder, np.float16).reshape(L, L, L))
    hxs = _shard_compact(
        np.asarray(h_dis_x, np.float16).reshape(L, L, L))
    hys = _shard_compact(
        np.asarray(h_dis_y, np.float16).reshape(L, L, L))
    # pack per-tile coefficient blocks: [NT, 3, TF]
    cf = np.stack([eds.reshape(NCORES, 128, NT, TF),
                   hxs.reshape(NCORES, 128, NT, TF),
                   hys.reshape(NCORES, 128, NT, TF)], axis=3)
    cf = np.ascontiguousarray(cf)          # (8, 128, NT, 3, TF)
    w = np.zeros((128, 2, 128), np.float16)
    d = np.arange(128)
    w[:, 0][d, d] = np.float16(j_v)
    w[:, 1][d, d] = np.float16(j_v * anis_v)
    cst = np.zeros((128, 8), np.float32)
    cst[:, 0] = np.sqrt(beta_v)
    cst[:, 1] = gamma_v
    return [
        {"x_in": xs[c], "p_in": ps[c], "cf_in": cf[c], "w_in": w,
         "cst_in": cst}
        for c in range(NCORES)
    ]


def assemble_output(results):
    """Per-core device outputs -> full (2N,) float32 array."""
    dxs = np.stack([results[c]["dx_out"] for c in range(NCORES)]
                   ).astype(np.float32)
    dps = np.stack([results[c]["dp_out"] for c in range(NCORES)]
                   ).astype(np.float32)
    dx3 = _unshard_compact(dxs)
    dp3 = _unshard_compact(dps)
    return np.concatenate([dx3.reshape(-1), dp3.reshape(-1)])


def kernel(t, y, J, anisotropy, gamma, h_dis_x, h_dis_y, beta, e_disorder,
           nn_idx_1, nn_idx_2, nn_idy_1, nn_idy_2, nn_idz_1, nn_idz_2):
    y = np.asarray(y, np.float32)
    J = np.asarray(J, np.float32)
    anisotropy = np.asarray(anisotropy, np.float32)
    gamma = np.asarray(gamma, np.float32)
    beta = np.asarray(beta, np.float32)
    h_dis_x = np.asarray(h_dis_x, np.float32)
    h_dis_y = np.asarray(h_dis_y, np.float32)
    e_disorder = np.asarray(e_disorder, np.float32)

    ok = (y.shape == (2 * N,)
          and _is_const(J) and _is_const(anisotropy)
          and _is_const(gamma) and _is_const(beta) and float(beta.flat[0]) >= 0
          and _rolls_ok(nn_idx_1, nn_idx_2, nn_idy_1, nn_idy_2,
                        nn_idz_1, nn_idz_2))
    if not ok:
        idx = [np.asarray(a) for a in (nn_idx_1, nn_idx_2, nn_idy_1,
                                       nn_idy_2, nn_idz_1, nn_idz_2)]
        return _numpy_fallback(y, J, anisotropy, gamma, h_dis_x, h_dis_y,
                               beta, e_disorder, idx)

    in_maps = prepare_in_maps(
        y, float(anisotropy.flat[0]), float(gamma.flat[0]),
        float(beta.flat[0]), float(J.flat[0]), h_dis_x, h_dis_y, e_disorder)
    res = _run(in_maps, trace=False)
    return assemble_output(res.results)


# revision 11
# speedup vs baseline: 2.1611x; 1.1251x over previous
"""Bass/Trainium2 kernel for the DGPE relaxation RHS on a 192^3 periodic lattice.

The nn_id* inputs are the deterministic 6-neighbor roll indices of the
lattice, so the gathers are implemented as stencil shifts.  The lattice is
sharded along axis 0 across 8 NeuronCores (24 planes + 2 halo planes each,
sliced host-side).  Within a core, partition = (k-block, j-block) = 8 x 16,
each partition holding a (24 x 12 x 24) sub-brick stored with j/k halo
strips so every neighbor access is a plain access-pattern offset.

v2: all device data is fp16 (DVE 2x mode, half DMA traffic).  The full
6-neighbor stencil runs on the idle PE as identity-weight matmuls over
shifted views accumulating in PSUM (J and J*anisotropy on the diagonals),
the squares / psum->sbuf copies / gamma scale run on ACT, leaving DVE
only the 12 irreducible two-source elementwise ops.
"""

import numpy as np

L = 192
N = L ** 3
NCORES = 8
CH = L // NCORES            # 24 planes (axis 0) per core
KH, JB = 8, 16              # partition grid: p = kh*JB + jb
JW = L // JB                # 12 j's per partition
KW = L // KH                # 24 k's per partition
IH = CH + 2                 # 26 planes incl. axis-0 halo
FJ = JW + 2                 # 14 incl. j halo strips
FK = KW + 2                 # 26 incl. k halo strips
PLF = FJ * FK               # padded plane free size (364)
FIN = IH * PLF
PF = JW * KW                # compact plane free size (288)
FOUT = CH * PF
T = 8                       # planes per compute tile
NT = CH // T
TF = T * PF

_STATE = {}


# ---------------------------------------------------------------- host side

def _shard_halo(v3pad):
    """(194,194,194) wrap-padded -> (8, 128, FIN) per-core images."""
    s0, s1, s2 = v3pad.strides
    v = np.lib.stride_tricks.as_strided(
        v3pad,
        shape=(NCORES, KH, JB, IH, FJ, FK),
        strides=(CH * s0, KW * s2, JW * s1, s0, s1, s2),
    )
    return np.ascontiguousarray(v).reshape(NCORES, 128, FIN)


def _shard_compact(v3):
    """(192,192,192) -> (8, 128, CH, PF) per-core compact images."""
    s0, s1, s2 = v3.strides
    v = np.lib.stride_tricks.as_strided(
        v3,
        shape=(NCORES, KH, JB, CH, JW, KW),
        strides=(CH * s0, KW * s2, JW * s1, s0, s1, s2),
    )
    return np.ascontiguousarray(v).reshape(NCORES, 128, CH, PF)


def _unshard_compact(per_core):
    """(8, 128, CH*PF) -> (192,192,192)."""
    out3 = np.empty((L, L, L), np.float32)
    s0, s1, s2 = out3.strides
    w = np.lib.stride_tricks.as_strided(
        out3,
        shape=(NCORES, KH, JB, CH, JW, KW),
        strides=(CH * s0, KW * s2, JW * s1, s0, s1, s2),
    )
    w[:] = per_core.reshape(NCORES, KH, JB, CH, JW, KW)
    return out3


def _is_const(a):
    a = np.asarray(a)
    return bool(a.size) and bool(np.all(a == a.flat[0]))


def _rolls_ok(nn_idx_1, nn_idx_2, nn_idy_1, nn_idy_2, nn_idz_1, nn_idz_2):
    """Spot-check that the index arrays are the periodic roll stencil."""
    rng = np.random.default_rng(12345)
    f = rng.integers(0, N, size=4096)
    i, r = np.divmod(f, L * L)
    j, k = np.divmod(r, L)

    def flat(ii, jj, kk):
        return (ii % L) * L * L + (jj % L) * L + (kk % L)

    checks = [
        (nn_idx_1, flat(i - 1, j, k)), (nn_idx_2, flat(i + 1, j, k)),
        (nn_idy_1, flat(i, j - 1, k)), (nn_idy_2, flat(i, j + 1, k)),
        (nn_idz_1, flat(i, j, k - 1)), (nn_idz_2, flat(i, j, k + 1)),
    ]
    for arr, want in checks:
        if not np.array_equal(np.asarray(arr)[f], want):
            return False
    return True


def _numpy_fallback(y, J, anisotropy, gamma, h_dis_x, h_dis_y, beta,
                    e_disorder, idx):
    """Exact reference math in numpy (used only if structure checks fail)."""
    x, p = y[:N], y[N:]

    def stencil(v):
        return J * (v[idx[0]] + v[idx[1]] + v[idx[2]] + v[idx[3]]
                    + anisotropy * (v[idx[4]] + v[idx[5]]))

    xL = stencil(x)
    yL = stencil(p)
    r2 = x * x + p * p
    cross = xL * p - yL * x
    dx = gamma * p * cross + e_disorder * p - yL + h_dis_y + beta * r2 * p
    dp = -gamma * x * cross - e_disorder * x + xL - h_dis_x - beta * r2 * x
    return np.concatenate([dx, dp]).astype(np.float32)


# -------------------------------------------------------------- device side

def _build_nc():
    from concourse import bacc
    import concourse.mybir as mybir
    from concourse.tile import TileContext

    ActF = mybir.ActivationFunctionType
    f32 = mybir.dt.float32
    f16 = mybir.dt.float16

    nc = bacc.Bacc("TRN2", target_bir_lowering=False, debug=False,
                   enable_asserts=False, num_devices=NCORES)
    x_in = nc.dram_tensor("x_in", [128, FIN], f16, kind="ExternalInput").ap()
    p_in = nc.dram_tensor("p_in", [128, FIN], f16, kind="ExternalInput").ap()
    # packed per-tile coefficients: [e_disorder | h_dis_x | h_dis_y]
    cf_in = nc.dram_tensor("cf_in", [128, NT, 3, TF], f16, kind="ExternalInput").ap()
    # stencil weights: [0] = J*I, [1] = J*anis*I
    w_in = nc.dram_tensor("w_in", [128, 2, 128], f16, kind="ExternalInput").ap()
    cst_in = nc.dram_tensor("cst_in", [128, 8], f32, kind="ExternalInput").ap()
    dx_out = nc.dram_tensor("dx_out", [128, FOUT], f16, kind="ExternalOutput").ap()
    dp_out = nc.dram_tensor("dp_out", [128, FOUT], f16, kind="ExternalOutput").ap()

    TPL = (T + 2) * PLF          # planes held per tile (incl. i halo)

    with TileContext(nc) as tc:
        with (
            tc.tile_pool(name="persist", bufs=1) as pers,
            tc.tile_pool(name="state", bufs=1) as sp,
            tc.tile_pool(name="coef", bufs=1) as cp,
            tc.tile_pool(name="lap", bufs=2) as lp,
            tc.tile_pool(name="scratch", bufs=2) as tp_,
            tc.tile_pool(name="outs", bufs=2) as op_,
            tc.psum_pool(name="ps", bufs=4) as pp,
        ):
            cst = pers.tile([128, 8], f32, name="cst")
            SQB = cst[:, 0:1]    # sqrt(beta)
            GAM = cst[:, 1:2]    # gamma
            w = pers.tile([128, 2, 128], f16, name="w")
            W1 = w[:, 0]
            W2 = w[:, 1]
            nc.sync.dma_start(cst[:], cst_in)
            nc.sync.dma_start(w[:], w_in)

            # PE pstate warmup: ~24 small matmuls while the first state
            # tiles stream in (ramps the PE clock toward 2.4 GHz)
            wps = pp.tile([128, 2, 512], f32, tag="ps", name="warm")
            for i in range(24):
                nc.tensor.matmul(wps[:, i % 2, :128], W1, W1,
                                 start=True, stop=True)

            # prefetch ALL inputs up front on the in-order SP queue so no
            # load descriptor ever sits behind a store that waits on compute
            xts, pts, cts = [], [], []
            for t in range(NT):
                i0 = t * T
                xt = sp.tile([128, TPL], f16, tag=f"xt{t}", name=f"xt{t}")
                nc.sync.dma_start(xt[:], x_in[:, i0 * PLF:(i0 + T + 2) * PLF])
                pt = sp.tile([128, TPL], f16, tag=f"pt{t}", name=f"pt{t}")
                nc.sync.dma_start(pt[:], p_in[:, i0 * PLF:(i0 + T + 2) * PLF])
                ct = cp.tile([128, 3, TF], f16, tag=f"ct{t}", name=f"ct{t}")
                nc.sync.dma_start(ct[:], cf_in[:, t])
                xts.append(xt)
                pts.append(pt)
                cts.append(ct)

            for t in range(NT):
                i0 = t * T
                f0 = i0 * PF
                xt, pt, ct = xts[t], pts[t], cts[t]
                ed, hx, hy = ct[:, 0, :], ct[:, 1, :], ct[:, 2, :]

                def sl(img, di, dj, dk, c0, cn):
                    """Shifted view of padded tile, planes [c0, c0+cn)."""
                    v = img[:].rearrange("q (i j k) -> q i j k",
                                         i=T + 2, j=FJ, k=FK)
                    return v[:, 1 + c0 + di: 1 + c0 + cn + di,
                             1 + dj: 1 + JW + dj, 1 + dk: 1 + KW + dk]

                xc = sl(xt, 0, 0, 0, 0, T)
                pc = sl(pt, 0, 0, 0, 0, T)

                def v4(tile):
                    return tile[:].rearrange("q (i j k) -> q i j k",
                                             i=T, j=JW, k=KW)

                # ---- stencil on PE: psum[c] = sum of 6 shifted views
                XL = lp.tile([128, TF], f16, tag="XL", name=f"XL{t}")
                YL = lp.tile([128, TF], f16, tag="YL", name=f"YL{t}")
                SHIFTS1 = ((-1, 0, 0), (1, 0, 0), (0, -1, 0), (0, 1, 0))
                SHIFTS2 = ((0, 0, -1), (0, 0, 1))
                for fi, (img, dst) in enumerate(((xt, XL), (pt, YL))):
                    for r in range(T // 2):
                        ps = pp.tile([128, 2, 512], f32, tag="ps",
                                     name=f"ps_{t}_{fi}_{r}")
                        for b in range(2):
                            c = r * 2 + b
                            po = ps[:, b, :PF]
                            for si, (di, dj, dk) in enumerate(SHIFTS1):
                                nc.tensor.matmul(
                                    po, W1, sl(img, di, dj, dk, c, 1),
                                    start=(si == 0), stop=False)
                            for si, (di, dj, dk) in enumerate(SHIFTS2):
                                nc.tensor.matmul(
                                    po, W2, sl(img, di, dj, dk, c, 1),
                                    start=False, stop=(si == 1))
                        nc.scalar.activation(
                            dst[:, r * 2 * PF:(r + 1) * 2 * PF].rearrange(
                                "q (b s) -> q b s", b=2),
                            ps[:, :, :PF], ActF.Copy)

                # ---- squares on ACT (after the psum copies in queue order:
                # the DVE chain needs XL/YL first, the squares only later)
                SQX = tp_.tile([128, TF], f16, tag="SQX", name=f"SQX{t}")
                SQP = tp_.tile([128, TF], f16, tag="SQP", name=f"SQP{t}")
                nc.scalar.activation(v4(SQX), xc, ActF.Square, scale=SQB)
                nc.scalar.activation(v4(SQP), pc, ActF.Square, scale=SQB)

                # ---- DVE chain (12 two-source ops + gamma 1-src)
                S1 = tp_.tile([128, TF], f16, tag="S1", name=f"S1_{t}")
                S2 = tp_.tile([128, TF], f16, tag="S2", name=f"S2_{t}")
                U1 = tp_.tile([128, TF], f16, tag="U1", name=f"U1_{t}")
                U2 = tp_.tile([128, TF], f16, tag="U2", name=f"U2_{t}")

                nc.vector.tensor_mul(v4(S1), v4(XL), pc)             # m1
                nc.vector.tensor_mul(v4(S2), v4(YL), xc)             # m2
                nc.vector.tensor_sub(S1[:], S1[:], S2[:])            # cross
                nc.vector.tensor_scalar_mul(S1[:], S1[:], GAM)       # * gamma
                nc.vector.tensor_sub(U1[:], hy, YL[:])               # u1
                nc.vector.tensor_sub(U2[:], XL[:], hx)               # u2
                nc.vector.tensor_add(S1[:], S1[:], ed)               # + ed
                nc.vector.tensor_add(S1[:], S1[:], SQX[:])           # + b x^2
                nc.vector.tensor_add(S1[:], S1[:], SQP[:])           # s2
                dxo = op_.tile([128, TF], f16, tag="dxo", name=f"dxo{t}")
                dpo = op_.tile([128, TF], f16, tag="dpo", name=f"dpo{t}")
                nc.vector.tensor_mul(v4(S2), pc, v4(S1))             # t1
                nc.vector.tensor_add(dxo[:], S2[:], U1[:])           # dx
                nc.sync.dma_start(dx_out[:, f0:f0 + TF], dxo[:])
                nc.vector.tensor_mul(v4(S2), xc, v4(S1))             # t2
                nc.vector.tensor_sub(dpo[:], U2[:], S2[:])           # dp
                nc.sync.dma_start(dp_out[:, f0:f0 + TF], dpo[:])

    nc.compile()
    return nc


def _get_nc():
    if "nc" not in _STATE:
        _STATE["nc"] = _build_nc()
    return _STATE["nc"]


def _run(in_maps, trace=False, trace_cores=None):
    from concourse.bass_utils import run_bass_kernel_spmd
    if trace:
        # the agent image's antenv lacks axon_hooks; wire the NTFF hook
        import sys as _sys
        import types as _types
        if "antenv.axon_hooks" not in _sys.modules:
            try:
                import trn_agent_boot.trn_boot as _tb
                _hook = _tb._ntff_profile_via_ctypes('/opt/axon/libaxon_pjrt.so')
                _mod = _types.ModuleType("antenv.axon_hooks")
                _mod.get_axon_ntff_profile_hook = lambda: _hook
                _sys.modules["antenv.axon_hooks"] = _mod
            except Exception:
                pass
    return run_bass_kernel_spmd(
        _get_nc(), in_maps, core_ids=list(range(NCORES)),
        trace=trace, trace_cores=trace_cores,
    )


def prepare_in_maps(y, anis_v, gamma_v, beta_v, j_v, h_dis_x, h_dis_y,
                    e_disorder):
    """Host-side sharding: build the 8 per-core input maps (fp16)."""
    x3 = np.asarray(y[:N], np.float32).reshape(L, L, L)
    p3 = np.asarray(y[N:], np.float32).reshape(L, L, L)
    xs = _shard_halo(np.pad(x3, 1, mode="wrap").astype(np.float16))
    ps = _shard_halo(np.pad(p3, 1, mode="wrap").astype(np.float16))
    eds = _shard_compact(
        np.asarray(e_disorder, np.float16).reshape(L, L, L))
    hxs = _shard_compact(
        np.asarray(h_dis_x, np.float16).reshape(L, L, L))
    hys = _shard_compact(
        np.asarray(h_dis_y, np.float16).reshape(L, L, L))
    # pack per-tile coefficient blocks: [NT, 3, TF]
    cf = np.stack([eds.reshape(NCORES, 128, NT, TF),
                   hxs.reshape(NCORES, 128, NT, TF),
                   hys.reshape(NCORES, 128, NT, TF)], axis=3)
    cf = np.ascontiguousarray(cf)          # (8, 128, NT, 3, TF)
    w = np.zeros((128, 2, 128), np.float16)
    d = np.arange(128)
    w[:, 0][d, d] = np.float16(j_v)
    w[:, 1][d, d] = np.float16(j_v * anis_v)
    cst = np.zeros((128, 8), np.float32)
    cst[:, 0] = np.sqrt(beta_v)
    cst[:, 1] = gamma_v
    return [
        {"x_in": xs[c], "p_in": ps[c], "cf_in": cf[c], "w_in": w,
         "cst_in": cst}
        for c in range(NCORES)
    ]


def assemble_output(results):
    """Per-core device outputs -> full (2N,) float32 array."""
    dxs = np.stack([results[c]["dx_out"] for c in range(NCORES)]
                   ).astype(np.float32)
    dps = np.stack([results[c]["dp_out"] for c in range(NCORES)]
                   ).astype(np.float32)
    dx3 = _unshard_compact(dxs)
    dp3 = _unshard_compact(dps)
    return np.concatenate([dx3.reshape(-1), dp3.reshape(-1)])


def kernel(t, y, J, anisotropy, gamma, h_dis_x, h_dis_y, beta, e_disorder,
           nn_idx_1, nn_idx_2, nn_idy_1, nn_idy_2, nn_idz_1, nn_idz_2):
    y = np.asarray(y, np.float32)
    J = np.asarray(J, np.float32)
    anisotropy = np.asarray(anisotropy, np.float32)
    gamma = np.asarray(gamma, np.float32)
    beta = np.asarray(beta, np.float32)
    h_dis_x = np.asarray(h_dis_x, np.float32)
    h_dis_y = np.asarray(h_dis_y, np.float32)
    e_disorder = np.asarray(e_disorder, np.float32)

    ok = (y.shape == (2 * N,)
          and _is_const(J) and _is_const(anisotropy)
          and _is_const(gamma) and _is_const(beta) and float(beta.flat[0]) >= 0
          and _rolls_ok(nn_idx_1, nn_idx_2, nn_idy_1, nn_idy_2,
                        nn_idz_1, nn_idz_2))
    if not ok:
        idx = [np.asarray(a) for a in (nn_idx_1, nn_idx_2, nn_idy_1,
                                       nn_idy_2, nn_idz_1, nn_idz_2)]
        return _numpy_fallback(y, J, anisotropy, gamma, h_dis_x, h_dis_y,
                               beta, e_disorder, idx)

    in_maps = prepare_in_maps(
        y, float(anisotropy.flat[0]), float(gamma.flat[0]),
        float(beta.flat[0]), float(J.flat[0]), h_dis_x, h_dis_y, e_disorder)
    res = _run(in_maps, trace=False)
    return assemble_output(res.results)
